# revision 1
# baseline (speedup 1.0000x reference)
"""Trainium2 Bass kernel for the linear-attention block (nn_Attention).

Per batch element (x: [256, 4096] after flattening h*w):
    qkv = w_qkv @ x; q,k,v heads of 64
    q = softmax_d(q) * 64**-0.5 ; k = softmax_n(k) ; v = v/4096
    ctx[h] = k[h] @ v[h].T ; out[h] = ctx[h].T @ q[h]
    y = w_out @ out + b_out ; LayerNorm_c(y) * g

Sharding: data-parallel over batch, 2 batch elements per core, no collectives.

Host folds (exact): v/n into w_v; q-scale into w_out; LN mean-centering into
w_out columns (so on-chip LN only needs sum(y^2)); softmax max-subtraction
skipped (inputs ~N(0,1), exp safe).

Layout: k,v computed transposed ([n, feat]) directly by the matmul so the
ctx contraction over n is a plain PE accumulation and ksum is a ones-matmul;
q stays original ([feat, n]), its per-head column sums via block-diag ones
matmul, broadcast back via a second ones matmul; all big reciprocals are
reciprocal_approx_fast; ACT runs only Exp + batched Sqrt (table reloads cost
1.3us each); y^2 on GpSimd.
"""

import numpy as np

HEADS = 4
DIM_HEAD = 64
SCALE = DIM_HEAD ** -0.5
EPS = 1e-5
B, C, H, W = 16, 256, 64, 64
N = H * W  # 4096
HID = HEADS * DIM_HEAD  # 256
NCORES = 8
BPC = B // NCORES  # batches per core = 2

NT = N // 512    # 8 n-tiles of 512
NCH = N // 128   # 32 n-chunks of 128
CT = C // 128    # 2 contraction tiles

_cache = {}


def _build_nc():
    import concourse.bass as bass
    import concourse.tile as tile
    from concourse import bacc, mybir

    f32 = mybir.dt.float32
    bf16 = mybir.dt.bfloat16
    AF = mybir.ActivationFunctionType
    OP = mybir.AluOpType

    nc = bacc.Bacc(None, target_bir_lowering=False, debug=False)
    x_ext = nc.declare_dram_parameter("x", [BPC, C, N], bf16, isOutput=False)
    wqkvT_ext = nc.declare_dram_parameter("wqkvT", [C, 3 * HID], bf16, isOutput=False)
    woutcT_ext = nc.declare_dram_parameter("woutcT", [HID, C], bf16, isOutput=False)
    bc_ext = nc.declare_dram_parameter("bc", [C, 1], f32, isOutput=False)
    g_ext = nc.declare_dram_parameter("g", [C, 1], f32, isOutput=False)
    onesbc_ext = nc.declare_dram_parameter("onesbc", [2, 128], bf16, isOutput=False)
    out_ext = nc.declare_dram_parameter("out", [BPC, C, N], f32, isOutput=True)

    with tile.TileContext(nc) as tc:
        with (
            tc.tile_pool(name="wts", bufs=1) as wts,
            tc.tile_pool(name="xs", bufs=2) as xs_pool,
            tc.tile_pool(name="qexp", bufs=2) as qexp_pool,
            tc.tile_pool(name="kv", bufs=4) as kv_pool,
            tc.tile_pool(name="small", bufs=4) as small_pool,
            tc.tile_pool(name="mid", bufs=4) as mid_pool,
            tc.tile_pool(name="fin", bufs=4) as fin_pool,
            tc.tile_pool(name="psum_mm", bufs=6, space="PSUM") as pmm,
            tc.tile_pool(name="psum_acc", bufs=1, space="PSUM") as pacc,
        ):
            # ---- constants & weights (loaded once, one DMA per tensor) ----
            wqkvT3 = wts.tile([128, CT, 3 * HID], bf16, tag="wqkvT", name="wqkvT")
            nc.sync.dma_start(out=wqkvT3, in_=wqkvT_ext[:, :].rearrange("(i p) o -> p i o", p=128))
            wqkvT = [wqkvT3[:, i] for i in range(CT)]
            woutcT3 = wts.tile([128, CT, C], bf16, tag="woutcTb", name="woutcTb")
            nc.sync.dma_start(out=woutcT3, in_=woutcT_ext[:, :].rearrange("(i p) o -> p i o", p=128))
            woutcT = [woutcT3[:, i] for i in range(CT)]
            bcg = wts.tile([128, 2, CT], f32, tag="bcg", name="bcg")
            nc.sync.dma_start(out=bcg[:, 0], in_=bc_ext[:, :].rearrange("(i p) o -> p (i o)", p=128))
            nc.sync.dma_start(out=bcg[:, 1], in_=g_ext[:, :].rearrange("(i p) o -> p (i o)", p=128))
            bc_sb = [bcg[:, 0, i:i + 1] for i in range(CT)]
            g_sb = [bcg[:, 1, i:i + 1] for i in range(CT)]

            ones128 = wts.tile([128, 1], bf16, tag="ones128", name="ones128")
            nc.vector.memset(ones128, 1.0)
            onesblk = wts.tile([128, 2], bf16, tag="onesblk", name="onesblk")
            nc.vector.memset(onesblk, 0.0)
            nc.vector.memset(onesblk[0:64, 0:1], 1.0)
            nc.vector.memset(onesblk[64:128, 1:2], 1.0)
            onesbc = wts.tile([2, 128], bf16, tag="onesbc", name="onesbc")
            nc.sync.dma_start(out=onesbc, in_=onesbc_ext[:, :])
            ones1 = wts.tile([1, 128], bf16, tag="ones1", name="ones1")
            nc.vector.memset(ones1, 1.0)
            eps_sb = wts.tile([128, 1], f32, tag="eps", name="eps")
            nc.vector.memset(eps_sb, EPS)

            for b in range(BPC):
                # ---- load x ----
                xs3 = xs_pool.tile([128, CT, N], bf16, tag="x", name="x")
                xr = x_ext[b].rearrange("(i p) n -> p i n", p=128)
                for q4 in range(4):
                    qsl = slice(q4 * (N // 4), (q4 + 1) * (N // 4))
                    nc.sync.dma_start(out=xs3[:, :, qsl], in_=xr[:, :, qsl])
                xs = [xs3[:, i] for i in range(CT)]

                # ---- stage C0 (hoisted): q matmul + exp, independent of k/v ----
                qexp = [qexp_pool.tile([128, N], bf16, tag=f"qexp{i}", name=f"qexp{i}") for i in range(2)]
                for nt in range(NT):
                    nsl = slice(nt * 512, (nt + 1) * 512)
                    for qt in range(2):
                        q_ps = pmm.tile([128, 512], f32, tag="mm", name="mm")
                        for ct in range(CT):
                            nc.tensor.matmul(
                                q_ps,
                                wqkvT[ct][:, qt * 128:(qt + 1) * 128],
                                xs[ct][:, nsl],
                                start=(ct == 0), stop=(ct == CT - 1),
                            )
                        nc.scalar.activation(out=qexp[qt][:, nsl], in_=q_ps, func=AF.Exp)

                # ---- stage A: kT/vT chunks + ksum + ctx accumulation ----
                ksum_t = pacc.tile([128, 2], f32, tag="ksum", name="ksum")
                ctx_t = pacc.tile([128, 256], f32, tag="ctx", name="ctx")
                ksum_ps = [ksum_t[:, i:i + 1] for i in range(CT)]
                ctx_ps = [ctx_t[:, i * 128:(i + 1) * 128] for i in range(2)]
                for nch in range(NCH):
                    kv_ps = pmm.tile([128, 512], f32, tag="mm", name="mm")
                    for ct in range(CT):
                        nc.tensor.matmul(
                            kv_ps,
                            xs[ct][:, nch * 128:(nch + 1) * 128],
                            wqkvT[ct][:, HID:3 * HID],
                            start=(ct == 0), stop=(ct == CT - 1),
                        )
                    kexp_t = kv_pool.tile([128, HID], bf16, tag="kexp", name="kexp")
                    nc.scalar.activation(out=kexp_t, in_=kv_ps[:, 0:HID], func=AF.Exp)
                    v_t = kv_pool.tile([128, HID], bf16, tag="v", name="v")
                    nc.vector.tensor_copy(out=v_t, in_=kv_ps[:, HID:2 * HID])
                    for i in range(CT):
                        # one bank holds both ksum chains: only the very first
                        # matmul clears has_written; later groups rely on the
                        # per-element has_written bits
                        nc.tensor.matmul(
                            ksum_ps[i],
                            kexp_t[:, i * 128:(i + 1) * 128],
                            ones128,
                            start=(nch == 0 and i == 0),
                            stop=(nch == NCH - 1 and i == CT - 1),
                            skip_group_check=True,
                        )
                    for hp in range(2):
                        nc.tensor.matmul(
                            ctx_ps[hp],
                            kexp_t[:, hp * 128:(hp + 1) * 128],
                            v_t[:, hp * 128:(hp + 1) * 128],
                            start=(nch == 0 and hp == 0),
                            stop=(nch == NCH - 1 and hp == 1),
                            skip_group_check=True,
                        )

                # ---- stage B: normalize ctx rows by 1/ksum (block-diag layout) ----
                krecip = [small_pool.tile([128, 1], f32, tag=f"krecip{i}", name=f"krecip{i}") for i in range(CT)]
                for i in range(CT):
                    nc.vector.reciprocal(out=krecip[i], in_=ksum_ps[i])
                ctx_sb = [small_pool.tile([128, 128], bf16, tag=f"ctxsb{i}", name=f"ctxsb{i}") for i in range(2)]
                for hp in range(2):
                    nc.vector.memset(ctx_sb[hp], 0.0)
                    for hh in range(2):
                        s = slice(hh * 64, hh * 64 + 64)
                        nc.vector.tensor_scalar(
                            out=ctx_sb[hp][s, s],
                            in0=ctx_ps[hp][s, s],
                            scalar1=krecip[hp][s],
                            scalar2=None,
                            op0=OP.mult,
                        )

                # ---- stage C: out = ctx^T @ q, y matmul ----
                y_sb = [fin_pool.tile([128, N], f32, tag=f"y{i}", name=f"y{i}", bufs=1) for i in range(2)]
                y2_sb = [fin_pool.tile([128, N], bf16, tag=f"y2{i}", name=f"y2{i}", bufs=1) for i in range(2)]
                for nt in range(NT):
                    nsl = slice(nt * 512, (nt + 1) * 512)
                    out_tiles = []
                    for qt in range(2):
                        qs_ps = pmm.tile([2, 512], f32, tag="mm", name="mm")
                        nc.tensor.matmul(qs_ps, onesblk, qexp[qt][:, nsl],
                                         start=True, stop=True)
                        qs_sb = small_pool.tile([2, 512], bf16, tag="qs", name="qs")
                        nc.vector.tensor_copy(out=qs_sb, in_=qs_ps)
                        qbc_ps = pmm.tile([128, 512], f32, tag="mm", name="mm")
                        nc.tensor.matmul(qbc_ps, onesbc, qs_sb, start=True, stop=True)
                        qbcr = mid_pool.tile([128, 512], f32, tag="qbcr", name="qbcr")
                        nc.vector.reciprocal_approx_fast(out=qbcr, in_=qbc_ps)
                        o_ps = pmm.tile([128, 512], f32, tag="mm", name="mm")
                        nc.tensor.matmul(o_ps, ctx_sb[qt], qexp[qt][:, nsl],
                                         start=True, stop=True)
                        out_sb = mid_pool.tile([128, 512], bf16, tag="outsb", name="outsb")
                        nc.vector.tensor_mul(out=out_sb, in0=o_ps, in1=qbcr)
                        out_tiles.append(out_sb)
                    for ot in range(2):
                        y_ps = pmm.tile([128, 512], f32, tag="mm", name="mm")
                        for et in range(2):
                            nc.tensor.matmul(
                                y_ps,
                                woutcT[et][:, ot * 128:(ot + 1) * 128],
                                out_tiles[et],
                                start=(et == 0), stop=(et == 1),
                            )
                        nc.vector.tensor_scalar_add(
                            out=y_sb[ot][:, nsl], in0=y_ps, scalar1=bc_sb[ot])
                        nc.gpsimd.tensor_mul(
                            out=y2_sb[ot][:, nsl], in0=y_sb[ot][:, nsl],
                            in1=y_sb[ot][:, nsl])
                # ---- stage D: LN epilogue (sqrts batched to limit ACT table loads) ----
                s2_sb = small_pool.tile([1, NT, 512], bf16, tag="s2", name="s2")
                sq_list = []
                for nt in range(NT):
                    nsl = slice(nt * 512, (nt + 1) * 512)
                    s2_ps = pmm.tile([1, 512], f32, tag="mm", name="mm")
                    for ot in range(2):
                        nc.tensor.matmul(s2_ps, ones128, y2_sb[ot][:, nsl],
                                         start=(ot == 0), stop=(ot == 1))
                    nc.vector.tensor_copy(out=s2_sb[:, nt], in_=s2_ps)
                for nt in range(NT):
                    s2bc_ps = pmm.tile([128, 512], f32, tag="mm", name="mm")
                    nc.tensor.matmul(s2bc_ps, ones1, s2_sb[:, nt], start=True, stop=True)
                    sq_sb = mid_pool.tile([128, 512], f32, tag="sq", name="sq")
                    nc.scalar.activation(out=sq_sb, in_=s2bc_ps, func=AF.Sqrt,
                                         bias=eps_sb, scale=1.0 / C)
                    sq_list.append(sq_sb)
                for nt in range(NT):
                    nsl = slice(nt * 512, (nt + 1) * 512)
                    rstd = mid_pool.tile([128, 512], f32, tag="rstd", name="rstd")
                    nc.vector.reciprocal_approx_fast(out=rstd, in_=sq_list[nt])
                    for ot in range(2):
                        fin = fin_pool.tile([128, 512], f32, tag="fin", name="fin")
                        nc.vector.scalar_tensor_tensor(
                            out=fin,
                            in0=y_sb[ot][:, nsl],
                            scalar=g_sb[ot],
                            in1=rstd,
                            op0=OP.mult,
                            op1=OP.mult,
                        )
                        nc.sync.dma_start(
                            out=out_ext[b, ot * 128:(ot + 1) * 128, nt * 512:(nt + 1) * 512], in_=fin
                        )
    nc.compile()
    return nc


def _prep_weights(w_qkv, w_out, b_out):
    import ml_dtypes
    w_qkv = np.asarray(w_qkv, dtype=np.float64)
    w_out = np.asarray(w_out, dtype=np.float64)
    b_out = np.asarray(b_out, dtype=np.float64)
    wq = w_qkv.copy()
    wq[2 * HID:3 * HID, :] /= N          # fold v/n
    wqkvT = np.ascontiguousarray(wq.T).astype(ml_dtypes.bfloat16)
    wo = w_out * SCALE                    # fold q scale
    wo = wo - wo.mean(axis=0, keepdims=True)  # fold LN mean-centering
    woutcT = np.ascontiguousarray(wo.T).astype(ml_dtypes.bfloat16)
    bc = (b_out - b_out.mean()).astype(np.float32).reshape(C, 1)
    return wqkvT, woutcT, bc


def kernel(x, w_qkv, w_out, b_out, g):
    import ml_dtypes
    from concourse.bass_utils import run_bass_kernel_spmd

    if "nc" not in _cache:
        _cache["nc"] = _build_nc()
    nc = _cache["nc"]

    xf = np.ascontiguousarray(np.asarray(x, dtype=np.float32).reshape(B, C, N).astype(ml_dtypes.bfloat16))
    wqkvT, woutcT, bc = _prep_weights(w_qkv, w_out, b_out)
    g2 = np.asarray(g, dtype=np.float32).reshape(C, 1)

    in_maps = []
    for i in range(NCORES):
        onesbc = np.zeros((2, 128), dtype=ml_dtypes.bfloat16)
        onesbc[0, 0:64] = 1.0
        onesbc[1, 64:128] = 1.0
        in_maps.append({
            "x": np.ascontiguousarray(xf[i * BPC:(i + 1) * BPC]),
            "wqkvT": wqkvT,
            "woutcT": woutcT,
            "bc": bc,
            "g": g2,
            "onesbc": onesbc,
        })
    res = run_bass_kernel_spmd(nc, in_maps, core_ids=list(range(NCORES)))
    outs = [res.results[i]["out"] for i in range(NCORES)]
    y = np.concatenate(outs, axis=0).reshape(B, C, H, W).astype(np.float32)
    return y



# revision 49
# speedup vs baseline: 1.4652x; 1.4652x over previous
"""Trainium2 Bass kernel for the linear-attention block (nn_Attention).

Per batch element (x: [256, 4096] after flattening h*w):
    qkv = w_qkv @ x; q,k,v heads of 64
    q = softmax_d(q) * 64**-0.5 ; k = softmax_n(k) ; v = v/4096
    ctx[h] = k[h] @ v[h].T ; out[h] = ctx[h].T @ q[h]
    y = w_out @ out + b_out ; LayerNorm_c(y) * g

Sharding: data-parallel over batch, 2 batch elements per core, no collectives.

Design (v4):
- k,v computed TRANSPOSED ([tok, feat]) per 128-token chunk so ctx is a PSUM
  accumulation over tokens; ksum rides along as a ones-column appended to the
  v tile (rhs [128,129] per head-pair) -> zero extra instructions for the
  k-softmax denominator.
- q computed NATURAL ([feat, n]); per-(head,n) colsums via block-ones matmul
  packed into ONE psum bank [32,512] at different partition offsets -> a
  single reciprocal instruction for the whole batch; broadcast back via a
  [2,128]-ones matmul; qn = qexp * qbc in one DVE op.
- G-fusion: G = ctx_norm @ w_out^T (per batch, tiny) replaces the separate
  out-projection matmul; LN bias is folded into G via the softmax row-sum
  property (sum_d qn = 1 per head), so y comes out of PSUM finished.
- y computed TRANSPOSED ([tok, c]) so LayerNorm-over-c is per-partition:
  variance via one STT square+accum per chunk, rstd = Exp(-0.5*Ln(var+eps))
  (Ln/Exp share the ACT table with Exp -> no table reloads), final scale via
  scalar_tensor_tensor with per-partition rstd and a broadcast g tile.
- Output written transposed [b, n, c] in bf16; host transposes back.
- Host folds (exact): v/n into w_v; q-scale + LN mean-centering into w_out.
- Engine balance per core: PE ~70us, ACT/DVE/GPS ~45-55us each.
"""

import numpy as np

HEADS = 4
DIM_HEAD = 64
SCALE = DIM_HEAD ** -0.5
EPS = 1e-5
B, C, H, W = 16, 256, 64, 64
N = H * W  # 4096
HID = HEADS * DIM_HEAD  # 256
NCORES = 8
BPC = B // NCORES  # batches per core = 2

NCH = N // 128   # 32 token chunks of 128
NT = N // 512    # 8 n-tiles of 512 for the q stage
CT = C // 128    # 2 contraction tiles

_cache = {}


def _build_nc(G_IS_FULL=False):
    import concourse.bass as bass
    import concourse.tile as tile
    from concourse import bacc, masks, mybir

    f32 = mybir.dt.float32
    bf16 = mybir.dt.bfloat16
    AF = mybir.ActivationFunctionType
    OP = mybir.AluOpType

    nc = bacc.Bacc(None, target_bir_lowering=False, debug=False)
    x_ext = nc.declare_dram_parameter("x", [BPC, 128, CT, N], bf16, isOutput=False)
    wqkvT_ext = nc.declare_dram_parameter("wqkvT", [128, CT, 3 * HID], bf16, isOutput=False)
    woutcT_ext = nc.declare_dram_parameter("woutcT", [128, CT, C], bf16, isOutput=False)
    bc4_ext = nc.declare_dram_parameter("bc4", [1, C], bf16, isOutput=False)
    g_ext = nc.declare_dram_parameter("g", [1, C], bf16, isOutput=False)
    # row-selector patterns for the qs broadcast matmuls (can't memset at
    # partition offsets other than 0/32/64)
    obc_ext = nc.declare_dram_parameter("obc", [8, 128], bf16, isOutput=False)
    out_ext = nc.declare_dram_parameter("out", [BPC, N, C], bf16, isOutput=True)

    with tile.TileContext(nc) as tc:
        with (
            tc.tile_pool(name="wts", bufs=1) as wts,
            tc.tile_pool(name="xs", bufs=2) as xs_pool,
            tc.tile_pool(name="kex", bufs=6) as kex_pool,
            tc.tile_pool(name="qb", bufs=1) as qb_pool,
            tc.tile_pool(name="qnb", bufs=2) as qn_pool,
            tc.tile_pool(name="small", bufs=3) as small_pool,
            tc.tile_pool(name="scr", bufs=6) as scr_pool,
            tc.tile_pool(name="stg", bufs=3) as stg_pool,
            tc.tile_pool(name="pbig", bufs=5, space="PSUM") as pbig,
            tc.tile_pool(name="pqs", bufs=1, space="PSUM") as pqs,
            tc.tile_pool(name="pctx", bufs=2, space="PSUM") as pctx,
        ):
            # ---- constants & weights (loaded once) ----
            wqkvT3 = wts.tile([128, CT, 3 * HID], bf16, tag="wqkvT", name="wqkvT")
            nc.sync.dma_start(out=wqkvT3, in_=wqkvT_ext[:, :, :])
            woutcT3 = wts.tile([128, CT, C], bf16, tag="woutcT", name="woutcT")
            nc.sync.dma_start(out=woutcT3, in_=woutcT_ext[:, :, :])
            bc4_sb = wts.tile([1, C], bf16, tag="bc4", name="bc4")
            nc.sync.dma_start(out=bc4_sb, in_=bc4_ext[:, :])
            g_row = wts.tile([1, C], bf16, tag="grow", name="grow")
            nc.sync.dma_start(out=g_row, in_=g_ext[:, :])

            ones1x128 = wts.tile([1, 128], bf16, tag="ones1x128", name="ones1x128")
            nc.vector.memset(ones1x128, 1.0)
            # qs lhsT pair: col j ones on partitions (j%2)*64.. ; a covers rows
            # 0,1 (qt=0), b covers rows 2,3 (qt=1) of the packed [4,512] sums
            onesblk4 = []
            for which in range(2):
                ob = wts.tile([128, 4], bf16, tag=f"onesblk4{which}", name=f"onesblk4{which}")
                nc.vector.memset(ob, 0.0)
                nc.vector.memset(ob[0:64, 2 * which:2 * which + 1], 1.0)
                nc.vector.memset(ob[64:128, 2 * which + 1:2 * which + 2], 1.0)
                onesblk4.append(ob)
            # qbc lhsT per qt: row 2qt -> partitions 0:64, row 2qt+1 -> 64:128
            onesbc4 = []
            for qt in range(2):
                ob = wts.tile([4, 128], bf16, tag=f"onesbc4{qt}", name=f"onesbc4{qt}")
                nc.sync.dma_start(out=ob, in_=obc_ext[4 * qt:4 * qt + 4, :])
                onesbc4.append(ob)
            eps_sb = wts.tile([128, 1], f32, tag="eps", name="eps")
            nc.vector.memset(eps_sb, EPS)

            ident = wts.tile([128, 128], bf16, tag="ident", name="ident")
            g_bc = wts.tile([128, C], bf16, tag="gbc", name="gbc")
            late = {"done": False}

            def build_late_consts():
                # deferred so the startup DMAs/PE aren't blocked by them
                if late["done"]:
                    return
                late["done"] = True
                masks.make_identity(nc, ident[:, :])
                if G_IS_FULL:
                    g_ps = pbig.tile([128, 512], f32, tag="big", name="gps")
                    nc.tensor.matmul(g_ps[:, 0:C], ones1x128, g_row,
                                     start=True, stop=True)
                    nc.vector.tensor_copy(out=g_bc, in_=g_ps[:, 0:C])

            # manual vT buffers holding 2 chunks each, with persistent ones
            # columns at 128/257/386/515 (the ksum column per head-pair)
            NVT = 4
            vts = []
            for i in range(NVT):
                vt = wts.tile([128, 516], bf16, tag=f"vt{i}", name=f"vt{i}")
                for j in range(4):
                    nc.vector.memset(vt[:, j * 129 + 128:j * 129 + 129], 1.0)
                vts.append(vt)

            # ---- load x for both batches (prefetch) ----
            xs_all = []
            for b in range(BPC):
                xs3 = xs_pool.tile([128, CT, N], bf16, tag="x", name="x")
                xr = x_ext[b]
                for q4 in range(4):
                    qsl = slice(q4 * (N // 4), (q4 + 1) * (N // 4))
                    nc.scalar.dma_start(out=xs3[:, :, qsl], in_=xr[:, :, qsl])
                xs_all.append([xs3[:, i] for i in range(CT)])

            # per-batch state carried between stages
            st = [dict() for _ in range(BPC)]

            def stage_AB(b, c_gen=None, at_cp1=None, at_end_ctx=None):
                """Interleaved kv+ctx (transposed) and q (natural) stages.
                Emission keeps the PE continuously busy so it ramps to the
                full p-state: kv/q matmuls depend only on x, while ctx/qs/qbc
                lag their producers by 1-2 iterations."""
                xs = xs_all[b]
                ctx_t = pctx.tile([128, 258], f32, tag="ctx", name="ctx")
                st[b]["ctx"] = ctx_t
                kexps = [None] * (NCH // 2)
                qexp = [qb_pool.tile([128, N], bf16, tag=f"qexp{qt}", name=f"qexp{qt}")
                        for qt in range(2)]
                qn = [qn_pool.tile([128, N], bf16, tag=f"qn{qt}", name=f"qn{qt}")
                      for qt in range(2)]
                st[b]["qn"] = qn
                qs_tiles = [None] * NT
                qsrs = [None] * NT

                def kv_mms(cp):
                    k2_ps = pbig.tile([128, 512], f32, tag="big", name="k2")
                    v2_ps = pbig.tile([128, 512], f32, tag="big", name="v2")
                    for half in range(2):
                        ch = cp * 2 + half
                        csl = slice(ch * 128, (ch + 1) * 128)
                        for ct in range(CT):
                            nc.tensor.matmul(
                                k2_ps[:, half * 256:(half + 1) * 256],
                                xs[ct][:, csl], wqkvT3[:, ct, HID:2 * HID],
                                start=(ct == 0), stop=(ct == CT - 1),
                                skip_group_check=True,
                            )
                        for ct in range(CT):
                            nc.tensor.matmul(
                                v2_ps[:, half * 256:(half + 1) * 256],
                                xs[ct][:, csl], wqkvT3[:, ct, 2 * HID:3 * HID],
                                start=(ct == 0), stop=(ct == CT - 1),
                                skip_group_check=True,
                            )
                    kexp = kex_pool.tile([128, 512], bf16, tag="kexp", name="kexp")
                    nc.scalar.activation(out=kexp, in_=k2_ps, func=AF.Exp)
                    kexps[cp] = kexp
                    vt = vts[cp % NVT]
                    vdst = vt.rearrange("p (hp x) -> p hp x", hp=4)[:, :, 0:128]
                    vsrc = v2_ps.rearrange("p (hp x) -> p hp x", hp=4)
                    nc.vector.tensor_copy(out=vdst, in_=vsrc)

                def ctx_mms(ch):
                    kex = kexps[ch // 2]
                    ko = (ch % 2) * 256
                    vt = vts[(ch // 2) % NVT]
                    vo = (ch % 2) * 258
                    for hp in range(2):
                        nc.tensor.matmul(
                            ctx_t[:, hp * 129:(hp + 1) * 129],
                            kex[:, ko + hp * 128:ko + (hp + 1) * 128],
                            vt[:, vo + hp * 129:vo + (hp + 1) * 129],
                            start=(ch == 0 and hp == 0),
                            stop=(ch == NCH - 1 and hp == 1),
                            skip_group_check=True,
                        )

                def q_mms(nt):
                    nsl = slice(nt * 512, (nt + 1) * 512)
                    for qt in range(2):
                        q_ps = pbig.tile([128, 512], f32, tag="big", name="q")
                        for ct in range(CT):
                            nc.tensor.matmul(
                                q_ps,
                                wqkvT3[:, ct, qt * 128:(qt + 1) * 128],
                                xs[ct][:, nsl],
                                start=(ct == 0), stop=(ct == CT - 1),
                            )
                        nc.scalar.activation(out=qexp[qt][:, nsl], in_=q_ps,
                                             func=AF.Exp)

                def qs_mms(nt):
                    nsl = slice(nt * 512, (nt + 1) * 512)
                    qs_ps = pqs.tile([4, 512], f32, tag="qs", name="qs")
                    qs_tiles[nt] = qs_ps
                    for qt in range(2):
                        nc.tensor.matmul(
                            qs_ps, onesblk4[qt], qexp[qt][:, nsl],
                            start=(qt == 0), stop=(qt == 1),
                        )

                def qsr_ops(nt):
                    qsrf = small_pool.tile([4, 512], f32, tag="qsrf", name="qsrf", bufs=2)
                    nc.vector.reciprocal_approx_fast(out=qsrf, in_=qs_tiles[nt])
                    qsr = small_pool.tile([4, 512], bf16, tag="qsr", name="qsr", bufs=4)
                    nc.scalar.activation(out=qsr, in_=qsrf, func=AF.Copy)
                    qsrs[nt] = qsr

                def qbc_qn(nt):
                    nsl = slice(nt * 512, (nt + 1) * 512)
                    for qt in range(2):
                        qbc_ps = pbig.tile([128, 512], f32, tag="big", name="qbc")
                        nc.tensor.matmul(qbc_ps, onesbc4[qt], qsrs[nt],
                                         start=True, stop=True)
                        nc.vector.tensor_mul(out=qn[qt][:, nsl],
                                             in0=qexp[qt][:, nsl], in1=qbc_ps)

                # software-pipelined emission: kv/q feed the PE immediately;
                # ctx lags kv by one pair; qs lags qexp; qbc lags qsr.
                for cp in range(NCH // 2):
                    kv_mms(cp)
                    if cp % 2 == 1:
                        nt = cp // 2
                        q_mms(nt)
                        if nt >= 1:
                            qs_mms(nt - 1)
                            qsr_ops(nt - 1)
                        if nt >= 3:
                            qbc_qn(nt - 3)
                    if cp == 1 and at_cp1 is not None:
                        at_cp1()
                    if cp >= 1:
                        ctx_mms(cp * 2 - 2)
                        ctx_mms(cp * 2 - 1)
                    if c_gen is not None and cp >= 2:
                        next(c_gen, None)
                ctx_mms(NCH - 2)
                ctx_mms(NCH - 1)
                if at_end_ctx is not None:
                    at_end_ctx()
                for nt in [NT - 1]:
                    qs_mms(nt)
                    qsr_ops(nt)
                for nt in [NT - 3, NT - 2, NT - 1]:
                    qbc_qn(nt)

            def stage_G_pre(b):
                """ctx normalize on DVE (krecip + block-diag scale)."""
                ctx_t = st[b]["ctx"]
                krecip = small_pool.tile([128, 2], f32, tag="krecip", name="krecip")
                for hp in range(2):
                    nc.vector.reciprocal(out=krecip[:, hp:hp + 1],
                                         in_=ctx_t[:, hp * 129 + 128:hp * 129 + 129])
                ctx_sb = [small_pool.tile([128, 128], bf16, tag=f"ctxsb{i}", name=f"ctxsb{i}")
                          for i in range(2)]
                for hp in range(2):
                    nc.vector.memset(ctx_sb[hp], 0.0)
                    for hh in range(2):
                        s = slice(hh * 64, hh * 64 + 64)
                        nc.vector.tensor_scalar(
                            out=ctx_sb[hp][s, s],
                            in0=ctx_t[s, hp * 129 + hh * 64:hp * 129 + hh * 64 + 64],
                            scalar1=krecip[s, hp:hp + 1],
                            scalar2=None,
                            op0=OP.mult,
                        )
                st[b]["ctx_sb"] = ctx_sb

            def stage_G_fin(b):
                """transpose + G = ctxT @ woutT (+ bias fold) on the PE."""
                ctx_sb = st[b]["ctx_sb"]
                G_sb = small_pool.tile([128, 2, C], bf16, tag="G", name="G")
                st[b]["G"] = G_sb
                for hp in range(2):
                    ctxT_ps = pbig.tile([128, 512], f32, tag="big", name="ctxT")
                    ctxT_ps_bf = ctxT_ps.bitcast(bf16)[:, 0:128]
                    nc.tensor.transpose(ctxT_ps_bf, ctx_sb[hp], ident)
                    ctxT_sb = small_pool.tile([128, 128], bf16, tag=f"ctxT{hp}", name=f"ctxT{hp}")
                    nc.vector.tensor_copy(out=ctxT_sb, in_=ctxT_ps_bf)
                    G_ps = pbig.tile([128, 512], f32, tag="big", name="Gps")
                    nc.tensor.matmul(G_ps[:, 0:C], ctxT_sb, woutcT3[:, hp],
                                     start=True, stop=False)
                    nc.tensor.matmul(G_ps[:, 0:C], ones1x128, bc4_sb,
                                     start=False, stop=True)
                    nc.vector.tensor_copy(out=G_sb[:, hp], in_=G_ps[:, 0:C])

            def stage_C(b, drain_on_act, sq_on_act=lambda ch: False):
                """yT = qn^T-chunks @ G (2 chunks per bank), per-token LN,
                write transposed out. Generator: yields after each pair.
                drain_on_act: put the PSUM->SBUF y drain on ACT (solo phase)
                or DVE (when interleaved with another batch's ACT-heavy AB)."""
                qn = st[b]["qn"]
                G_sb = st[b]["G"]
                s2_all = small_pool.tile([128, NCH], f32, tag="s2", name="s2")
                rstd_all = small_pool.tile([128, NCH], f32, tag="rstd", name="rstd")
                outr = out_ext[b].rearrange("(c p) f -> p c f", p=128)
                y_tiles = [None] * 4
                stg = None
                for g4 in range(NCH // 4):
                    for cp in range(2):
                        ch0 = g4 * 4 + cp * 2
                        if ch0 % 8 == 0:
                            stg = stg_pool.tile([128, 8, C], bf16, tag="stg", name="stg")
                        yT2_ps = pbig.tile([128, 512], f32, tag="big", name="yT2")
                        for half in range(2):
                            ch = ch0 + half
                            csl = slice(ch * 128, (ch + 1) * 128)
                            for qt in range(2):
                                nc.tensor.matmul(
                                    yT2_ps[:, half * C:(half + 1) * C],
                                    qn[qt][:, csl], G_sb[:, qt],
                                    start=(qt == 0), stop=(qt == 1),
                                    skip_group_check=True,
                                )
                        y_sb2 = scr_pool.tile([128, 512], bf16, tag="ysb2", name="ysb2")
                        if drain_on_act(ch0):
                            nc.scalar.activation(out=y_sb2, in_=yT2_ps, func=AF.Copy)
                        else:
                            nc.vector.tensor_copy(out=y_sb2, in_=yT2_ps)
                        y_tiles[cp * 2] = (y_sb2, 0, stg, ch0)
                        y_tiles[cp * 2 + 1] = (y_sb2, 1, stg, ch0 + 1)
                        for half in range(2):
                            ch = ch0 + half
                            y2scr = scr_pool.tile([128, C], bf16, tag="y2", name="y2")
                            if sq_on_act(ch):
                                nc.scalar.activation(
                                    out=y2scr,
                                    in_=y_sb2[:, half * C:(half + 1) * C],
                                    func=AF.Square,
                                    accum_out=s2_all[:, ch:ch + 1],
                                )
                            else:
                                nc.vector.scalar_tensor_tensor(
                                    out=y2scr, in0=y_sb2[:, half * C:(half + 1) * C],
                                    scalar=1.0, in1=y_sb2[:, half * C:(half + 1) * C],
                                    op0=OP.mult, op1=OP.mult,
                                    accum_out=s2_all[:, ch:ch + 1],
                                )
                        yield
                    gsl = slice(g4 * 4, g4 * 4 + 4)
                    sqv = scr_pool.tile([128, 4], f32, tag="sqv", name="sqv")
                    nc.scalar.activation(out=sqv, in_=s2_all[:, gsl], func=AF.Sqrt,
                                         bias=eps_sb, scale=1.0 / C)
                    nc.vector.reciprocal_approx_fast(out=rstd_all[:, gsl], in_=sqv)
                    for i in range(4):
                        y_sb2, half, stg_t, ch = y_tiles[i]
                        ysl = y_sb2[:, half * C:(half + 1) * C]
                        nc.vector.tensor_scalar(
                            out=stg_t[:, ch % 8], in0=ysl,
                            scalar1=rstd_all[:, ch:ch + 1], scalar2=None,
                            op0=OP.mult)
                        if G_IS_FULL:
                            nc.gpsimd.tensor_mul(out=stg_t[:, ch % 8],
                                                 in0=stg_t[:, ch % 8], in1=g_bc)
                    nc.sync.dma_start(
                        out=outr[:, g4 * 4:(g4 + 1) * 4],
                        in_=stg[:, (g4 % 2) * 4:(g4 % 2) * 4 + 4])

            # emission order: both batches' matmul-dense stages back-to-back
            # keep the PE continuously busy (p-state ramp); the G chains hide
            # under the other batch's matmul work.
            stage_AB(0)
            build_late_consts()
            stage_G_pre(0)
            c0 = stage_C(0, drain_on_act=lambda ch: False)
            stage_AB(1, c_gen=c0, at_cp1=lambda: stage_G_fin(0))
            stage_G_pre(1)
            for _ in c0:
                pass
            stage_G_fin(1)
            for _ in stage_C(1, drain_on_act=lambda ch: True,
                             sq_on_act=lambda ch: ch % 4 == 0):
                pass

    nc.compile()
    return nc


def _prep_weights(w_qkv, w_out, b_out, g):
    import ml_dtypes
    w_qkv = np.asarray(w_qkv, dtype=np.float64)
    w_out = np.asarray(w_out, dtype=np.float64)
    b_out = np.asarray(b_out, dtype=np.float64)
    g64 = np.asarray(g, dtype=np.float64)
    wq = w_qkv.copy()
    wq[2 * HID:3 * HID, :] /= N          # fold v/n
    wqkvT = wq.T.reshape(CT, 128, 3 * HID).transpose(1, 0, 2)
    wqkvT = np.ascontiguousarray(wqkvT).astype(ml_dtypes.bfloat16)
    wo = w_out * SCALE                    # fold q scale
    wo = wo - wo.mean(axis=0, keepdims=True)  # fold LN mean-centering
    woutcT = wo.T.reshape(CT, 128, C).transpose(1, 0, 2)
    woutcT = np.ascontiguousarray(woutcT).astype(ml_dtypes.bfloat16)
    bc4 = ((b_out - b_out.mean()) / 4.0).astype(ml_dtypes.bfloat16).reshape(1, C)
    g_row = g64.astype(ml_dtypes.bfloat16).reshape(1, C)
    return wqkvT, woutcT, bc4, g_row


def _make_in_maps(x, w_qkv, w_out, b_out, g):
    import ml_dtypes
    xf = np.asarray(x, dtype=np.float32).reshape(B, CT, 128, N).transpose(0, 2, 1, 3)
    xf = np.ascontiguousarray(xf).astype(ml_dtypes.bfloat16)
    wqkvT, woutcT, bc4, g_row = _prep_weights(w_qkv, w_out, b_out, g)
    obc = np.zeros((8, 128), dtype=ml_dtypes.bfloat16)
    for qt in range(2):
        obc[4 * qt + 2 * qt, 0:64] = 1.0
        obc[4 * qt + 2 * qt + 1, 64:128] = 1.0
    in_maps = []
    for i in range(NCORES):
        in_maps.append({
            "x": np.ascontiguousarray(xf[i * BPC:(i + 1) * BPC]),
            "wqkvT": wqkvT,
            "woutcT": woutcT,
            "bc4": bc4,
            "g": g_row,
            "obc": obc,
        })
    return in_maps


def kernel(x, w_qkv, w_out, b_out, g):
    from concourse.bass_utils import run_bass_kernel_spmd

    g_full = not np.allclose(np.asarray(g, dtype=np.float64), 1.0)
    key = f"nc{int(g_full)}"
    if key not in _cache:
        _cache[key] = _build_nc(G_IS_FULL=g_full)
    nc = _cache[key]

    in_maps = _make_in_maps(x, w_qkv, w_out, b_out, g)
    res = run_bass_kernel_spmd(nc, in_maps, core_ids=list(range(NCORES)))
    outs = [res.results[i]["out"] for i in range(NCORES)]
    yT = np.concatenate(outs, axis=0).astype(np.float32)  # [B, N, C]
    y = np.ascontiguousarray(yT.transpose(0, 2, 1)).reshape(B, C, H, W)
    return y
